# revision 1
# baseline (speedup 1.0000x reference)
"""Multi-head self-attention Trainium2 Bass kernel, v2.

Same sharding as v1 (core = batch x head-group, host sums the two
head-group partial projections), restructured for engine overlap:

- DMA order: biases/id, w_qk(pair0), x_t, w_v -- the PE starts QK(pair0)
  as soon as x_t chunks land instead of idling ~28us at the start.
- Flipped AV: U[q,d] = E^T[k,q]^T V[k,d] with the exp tile E^T as the
  *stationary* operand and V_h as the 64-wide moving operand (plus an
  N=1 ones column for the softmax denominator).  Streams 65 cols per
  128-key chunk instead of 1024: AV drops from ~109us to ~56us of PE.
- Work injection: V chunks run inside pair0's first window, QK(p+1)
  inside pair p, transposes of pair p inside pair p+1, and the output
  projection of the first two token windows inside pair 3 -- the
  Activation engine (~266us of exp) is fed continuously.
- y comes out of the flipped AV as [token, feat]; PE transpose matmuls
  (vs a bf16 identity) restore [feat, token] for the projection.
"""

from collections import deque
from contextlib import ExitStack

import bass_rust as _br


def _order_after(inst, first):
    deps = _br.InstructionNameOrderedSet()
    deps.add(first.ins.name)
    inst.ins.add_nosync_dependencies_from(deps)

import numpy as np
import ml_dtypes

import concourse.bass as bass
import concourse.bacc as bacc
import concourse.tile as tile
from concourse import mybir
from concourse.bass_utils import run_bass_kernel_spmd

N_CORES = 8
C = 1024           # embed dim
H = 16             # total heads
HD = 64            # head dim
HPC = 8            # heads per core
CG = HPC * HD      # 512: per-core q/k/v feature width

F32 = mybir.dt.float32
F32R = mybir.dt.float32r
BF16 = mybir.dt.bfloat16


def _body(tc, T, x_t, w_qk, b_qk, w_v, w_p, b_out, id_bf, out_t):
    nc = tc.nc
    KC = C // 128            # 8 contraction chunks over C
    FC = 2 * CG // 128       # 8 q||k feature chunks
    TC1 = T // 128           # key/token chunks of 128
    QW = min(1024, T)        # query window
    NW = T // QW             # windows per pair
    NQ = QW // 128           # query chunks per window
    T4 = max(1, T // 512)    # output token windows
    OCC = C // 128           # output channel chunks
    PCH = CG // 128          # proj contraction chunks
    NPAIR = HPC // 2
    full = (NPAIR == 4 and T4 == 4 and NW == 2)
    Exp = mybir.ActivationFunctionType.Exp
    Mult = mybir.AluOpType.mult

    with ExitStack() as stack:
        constp = stack.enter_context(tc.tile_pool(name="const", bufs=1))
        pers = stack.enter_context(tc.tile_pool(name="persist", bufs=1))
        wqkp = stack.enter_context(tc.tile_pool(name="wqk", bufs=4))
        ep = stack.enter_context(tc.tile_pool(name="e", bufs=4))
        recp = stack.enter_context(tc.tile_pool(name="rec", bufs=2))
        yp = stack.enter_context(tc.tile_pool(name="y", bufs=3))
        outp = stack.enter_context(tc.tile_pool(name="outp", bufs=2))
        psp = stack.enter_context(tc.tile_pool(name="ps", bufs=1, space="PSUM"))
        wv_stack = ExitStack()
        wvp = wv_stack.enter_context(tc.tile_pool(name="wv", bufs=1))

        # ---- constants / small inputs first in DMA queue order
        bqk_sb = constp.tile([128, FC], F32, tag="bqk")
        nc.sync.dma_start(bqk_sb[:], b_qk[:])
        bout_sb = constp.tile([128, OCC], F32, tag="bout")
        nc.sync.dma_start(bout_sb[:], b_out[:])
        id_sb = constp.tile([128, 128], BF16, tag="id")
        ones_sb = constp.tile([128, 1], BF16, tag="ones")
        nc.vector.memset(ones_sb[:], 1.0)

        # w_qk chunks: pair-0's Q/K chunks go before x_t in the queue
        wqk_tiles = {}

        def load_wqk(fc):
            t = wqkp.tile([128, KC * 128], BF16, tag="wqkfc",
                          name=f"wqkfc_{fc}")
            nc.sync.dma_start(
                t.rearrange("p (k c) -> p k c", c=128)[:],
                w_qk[:, fc * 128:(fc + 1) * 128]
                .rearrange("(k p) c -> p k c", p=128))
            wqk_tiles[fc] = [t[:, kc * 128:(kc + 1) * 128]
                             for kc in range(KC)]

        load_wqk(0)
        load_wqk(NPAIR)

        # activations: [C, T] f32.  Each chunk's DMA is split in half
        # along T and ordered first-halves -> w_v -> second-halves, so the
        # first attention window's QK blocks and V chunks are ready ~20us
        # earlier than a monolithic in-order load.
        TH = T // 2
        xt = []
        for i in range(KC):
            t = pers.tile([128, T], BF16, tag=f"xt{i}")
            nc.sync.dma_start(t[:, 0:TH], x_t[i * 128:(i + 1) * 128, 0:TH])
            xt.append(t)
        wv_all = wvp.tile([128, KC * CG], BF16, tag="wvall")
        nc.sync.dma_start(
            wv_all.rearrange("p (k c) -> p k c", c=CG)[:],
            w_v.rearrange("(k p) c -> p k c", p=128))
        wv = [wv_all[:, kc * CG:(kc + 1) * CG] for kc in range(KC)]
        for i in range(KC):
            nc.sync.dma_start(xt[i][:, TH:T], x_t[i * 128:(i + 1) * 128, TH:T])
        # identity (for the PE transposes) isn't needed until pair 1
        nc.sync.dma_start(id_sb[:], id_bf[:])

        # ---- persistent intermediates
        # bf16 q/k activations: halves SBUF and keeps PE at full rate
        qkt = [pers.tile([128, T], BF16, tag=f"qkt{i}", name=f"qkt{i}")
               for i in range(FC)]
        v2 = [pers.tile([128, CG], BF16, tag=f"v2_{i}", name=f"v2_{i}")
              for i in range(TC1)]
        # yt[fcp]: [128 feat, T] for the projection
        yt = [pers.tile([128, T], BF16, tag=f"yt{i}", name=f"yt{i}")
              for i in range(PCH)]
        # y[pair][s]: [128 tok, 64 feat] chunks packed [win*512 + qi*64+d],
        # double-buffered across pairs (pair p is transposed during p+1)
        ysb = {}
        # output staging: one [128, T] bf16 tile per output-channel chunk,
        # written per token window (some injected into pair 3), one DMA each
        osbs = [pers.tile([128, T], BF16, tag=f"osb{occ}", name=f"osb{occ}")
                for occ in range(OCC)]
        wp = []

        def load_wp():
            wpp = stack.enter_context(tc.tile_pool(name="wp", bufs=1))
            t = wpp.tile([128, PCH * C], BF16, tag="wpall")
            nc.sync.dma_start(
                t.rearrange("p (f c) -> p f c", c=C)[:],
                w_p.rearrange("(f p) c -> p f c", p=128))
            for fcp in range(PCH):
                wp.append(t[:, fcp * C:(fcp + 1) * C])

        # ---- PSUM tiles (8 banks exactly)
        sps = [psp.tile([128, QW], F32, tag=f"sps{s}", name=f"sps{s}")
               for s in range(2)]
        ups = [psp.tile([128, NQ * 64], F32, tag=f"ups{s}",
                        name=f"ups{s}") for s in range(2)]
        # full bank so no other tile shares den's zero region
        den = psp.tile([128, 512], F32, tag="den")
        trans = psp.tile([128, 512], F32, tag="trans")

        # ---------- injected work items ----------
        def qk_block(fc, t4w, half, bank=None, quarter=None):
            """Half (or quarter) of a QK sub-block: contraction matmuls
            into a psum bank (trans by default); bias added via DVE after
            the last chunk."""
            ps = trans[:, 0:512] if bank is None else bank
            rng = (range(quarter * 2, quarter * 2 + 2) if quarter is not None
                   else range(half * 4, half * 4 + 4))
            for kc in rng:
                nc.tensor.matmul(
                    ps[:], wqk_tiles[fc][kc],
                    xt[kc][:, t4w * 512:(t4w + 1) * 512],
                    start=(kc == 0), stop=(kc == KC - 1))
            if (half == 1 if quarter is None else quarter == 3):
                nc.vector.tensor_scalar_add(
                    qkt[fc][:, t4w * 512:(t4w + 1) * 512], ps[:],
                    bqk_sb[:, fc:fc + 1])

        def v_chunk(tokc):
            """One V chunk: 8 contraction matmuls + copy to bf16."""
            for kc in range(KC):
                nc.tensor.matmul(
                    trans[:, 0:CG], xt[kc][:, tokc * 128:(tokc + 1) * 128],
                    wv[kc], start=(kc == 0), stop=(kc == KC - 1))
            nc.vector.tensor_copy(v2[tokc][:], trans[:, 0:CG])

        def transp_win(pair, t4w):
            """Transpose y[pair] tokens t4w*512.. into yt[pair] via PE."""
            tp = trans.bitcast(BF16)
            nj = min(4, TC1 - t4w * 4)
            for s in range(2):
                for j in range(nj):
                    tc_idx = t4w * 4 + j
                    win, qi = tc_idx // NQ, tc_idx % NQ
                    nc.tensor.matmul(
                        tp[s * 64:(s + 1) * 64, j * 128:(j + 1) * 128],
                        ysb[pair][s][:, win * (NQ * 64) + qi * 64:
                                     win * (NQ * 64) + qi * 64 + 64],
                        id_sb[:], start=True, stop=True, is_transpose=True)
            w = min(512, T - t4w * 512)
            nc.vector.tensor_copy(
                yt[pair][:, t4w * 512:t4w * 512 + w], tp[:, 0:w])

        def proj_occ(occ, t4w, bank=None, dma=False):
            """One output-channel chunk of the projection for window t4w.
            bank: psum [128, 512] target; defaults to trans (serial)."""
            ps = trans[:, 0:512] if bank is None else bank
            w = min(512, T - t4w * 512)
            osb = osbs[occ]
            for fcp in range(PCH):
                nc.tensor.matmul(
                    ps[:, 0:w], wp[fcp][:, occ * 128:(occ + 1) * 128],
                    yt[fcp][:, t4w * 512:t4w * 512 + w],
                    start=(fcp == 0), stop=(fcp == PCH - 1))
            nc.vector.tensor_scalar_add(
                osb[:, t4w * 512:t4w * 512 + w], ps[:, 0:w],
                bout_sb[:, occ:occ + 1])
            if dma:
                nc.sync.dma_start(out_t[occ * 128:(occ + 1) * 128, :],
                                  osb[:])

        # ---------- upfront: QK pair 0 only ----------
        # only the blocks gating the first exp run ahead of the pair loop:
        # Q window 0 and K keys 0:1024 (first-half x_t).  Everything else
        # (V chunks included) is injected into window 0 so the in-order PE
        # queue isn't stalled on the w_v / second-half x_t DMAs.
        # rotate the upfront blocks over four banks: a single bank would
        # serialize each block behind the previous block's DVE bias-read.
        sbanks = [trans[:, 0:512], den[:, 0:512]]
        if NQ * 64 >= 512:
            sbanks += [ups[0][:, 0:512], ups[1][:, 0:512]]
        half_t4 = max(1, T4 // 2)
        sbi = 0
        for fc in (0, NPAIR):
            for t4w in range(half_t4):
                b = sbanks[sbi % len(sbanks)]
                sbi += 1
                qk_block(fc, t4w, 0, bank=b)
                qk_block(fc, t4w, 1, bank=b)
        if not full:
            v_chunk(0)
            if TC1 > 1:
                v_chunk(1)

        # ---------- injection schedule ----------
        schedules = {(p, w): deque() for p in range(NPAIR)
                     for w in range(NW)}

        def sched_items(slots, items):
            """Spread items round-robin over the given (pair,win) slots."""
            per = (len(items) + len(slots) - 1) // len(slots)
            it = iter(items)
            for slot in slots:
                for _ in range(per):
                    try:
                        schedules[slot].append(next(it))
                    except StopIteration:
                        return

        def sched_front(slot, items):
            schedules[slot].extendleft(reversed(items))

        def qk_items(fcs, quarters=False):
            out = []
            for fc in fcs:
                for t4w in range(T4):
                    if quarters:
                        for q in range(4):
                            out.append(lambda fc=fc, t4w=t4w, q=q:
                                       qk_block(fc, t4w, 0, quarter=q))
                    else:
                        for half in range(2):
                            out.append(lambda fc=fc, t4w=t4w, half=half:
                                       qk_block(fc, t4w, half))
            return out

        if full:
            # pair0 win0 (two pops per iteration): V chunks 2..15
            # interleaved with pair0's remaining K-then-Q blocks, ordered
            # so everything is ready just before its first consumer.
            rest = []
            for fc in (NPAIR, 0):
                for t4w in range(half_t4, T4):
                    for half in range(2):
                        rest.append(lambda fc=fc, t4w=t4w, half=half:
                                    qk_block(fc, t4w, half))
            vitems = [lambda tokc=tokc: v_chunk(tokc)
                      for tokc in range(2, TC1)]
            # V chunks lead (v2[kc] is read at iteration kc; two pops per
            # iteration keep them ahead); the K blocks for keys 1024:2048
            # (gated on the second-half x_t DMAs) slot in around iteration
            # 5 so they neither stall the in-order PE queue nor miss their
            # first use at iteration 8.
            vitems = [lambda tokc=tokc: v_chunk(tokc)
                      for tokc in range(TC1)]
            mix = vitems[:10] + rest[:4] + vitems[10:14] + rest[4:] \
                + vitems[14:]
            sched_items([(0, 0)], mix)
            load_wqk(1)
            load_wqk(NPAIR + 1)
            sched_items([(0, 1)], qk_items([1, NPAIR + 1]))
            load_wqk(2)
            load_wqk(NPAIR + 2)
            sched_items([(1, 0), (1, 1)],
                        qk_items([2, NPAIR + 2], quarters=True))
            load_wqk(3)
            load_wqk(NPAIR + 3)
            sched_items([(2, 0), (2, 1)],
                        qk_items([3, NPAIR + 3], quarters=True))
            sched_items([(3, 0)],
                        [lambda t4w=t4w: transp_win(0, t4w)
                         for t4w in range(T4)]
                        + [lambda t4w=t4w: transp_win(1, t4w)
                           for t4w in range(T4)]
                        + [lambda t4w=t4w: transp_win(2, t4w)
                           for t4w in range(T4)])
            win1_items = ([lambda: transp_win(3, 0),
                           lambda: transp_win(3, 1)]
                          + [lambda occ=occ, t4w=t4w: proj_occ(occ, t4w)
                             for t4w in (0, 1) for occ in range(OCC)])
            tail_extra = win1_items[16:]
            sched_items([(3, 1)], win1_items[:16])
        else:
            # small configs: load everything up front, no injection
            for p in range(1, NPAIR):
                load_wqk(p)
                load_wqk(NPAIR + p)
            sched_items([(0, 0)],
                        [lambda tokc=tokc: v_chunk(tokc)
                         for tokc in range(2, TC1)])

        # ---------- the attention pair loop ----------
        for pair in range(NPAIR):
            if pair == 1:
                wv_stack.close()   # V done: free w_v, then load w_p
                load_wp()
            ysb[pair] = [yp.tile([128, T // 2], BF16, tag=f"y{s}",
                                 name=f"y{pair}_{s}") for s in range(2)]
            if not full and pair > 0:
                for fc in (pair, NPAIR + pair):
                    for t4w in range(T4):
                        qk_block(fc, t4w, 0)
                        qk_block(fc, t4w, 1)
            qt, kt = qkt[pair], qkt[NPAIR + pair]
            for win in range(NW):
                q0 = win * QW
                queue = schedules[(pair, win)]
                for kc in range(TC1):
                    first, last = (kc == 0), (kc == TC1 - 1)
                    ets = {}
                    for s in range(2):
                        po = s * 64
                        for j in range(QW // 512):
                            nc.tensor.matmul(
                                sps[s][:, j * 512:(j + 1) * 512],
                                kt[po:po + 64, kc * 128:(kc + 1) * 128],
                                qt[po:po + 64,
                                   q0 + j * 512:q0 + (j + 1) * 512],
                                start=True, stop=True)
                        et = ep.tile([128, QW], BF16, tag=f"et{s}",
                                     name=f"et{s}_{pair}_{win}_{kc}")
                        nc.scalar.activation(et[:], sps[s][:], Exp,
                                             scale=0.125)
                        ets[s] = et
                    # injected work sits in the act-wait shadow: the AV
                    # matmuls below can't start until the exps finish.
                    npop = 2 if (full and pair == 0 and win == 0) else 1
                    for _ in range(min(npop, len(queue))):
                        queue.popleft()()
                    # one start/stop per psum bank per window: a group
                    # start zeroes the full 2KB zero region, so only the
                    # first matmul into each bank may carry start=True and
    # it must be ordered before the bank's other kc==0 writes
                    # (their ranges are disjoint, so data deps don't).
                    den_start = None
                    for s in range(2):
                        h = 2 * pair + s
                        ups_start = None
                        for qi in range(NQ):
                            etc = ets[s][:, qi * 128:(qi + 1) * 128]
                            mu = nc.tensor.matmul(
                                ups[s][:, qi * 64:(qi + 1) * 64], etc,
                                v2[kc][:, h * 64:(h + 1) * 64],
                                start=(first and qi == 0),
                                stop=(last and qi == NQ - 1))
                            md = nc.tensor.matmul(
                                den[:, s * NQ + qi:s * NQ + qi + 1], etc,
                                ones_sb[:],
                                start=(first and s == 0 and qi == 0),
                                stop=(last and s == 1 and qi == NQ - 1))
                            if first:
                                if qi == 0:
                                    ups_start = mu
                                else:
                                    _order_after(mu, ups_start)
                                if s == 0 and qi == 0:
                                    den_start = md
                                else:
                                    _order_after(md, den_start)
                while queue:       # drain leftovers (small-T configs)
                    queue.popleft()()
                # normalize both heads of this window
                rden = recp.tile([128, 2 * NQ], F32, tag="rden",
                                 name=f"rden_{pair}_{win}")
                nc.vector.reciprocal(rden[:], den[:, 0:2 * NQ])
                for s in range(2):
                    yv = ysb[pair][s].rearrange("p (w q d) -> p w q d",
                                                w=NW, d=64)
                    uv = ups[s].rearrange("p (q d) -> p q d", d=64)
                    nc.vector.tensor_tensor(
                        yv[:, win, :, :], uv[:],
                        rden[:, s * NQ:(s + 1) * NQ].unsqueeze(2)
                        .to_broadcast((128, NQ, 64)),
                        op=Mult)
            if not full:
                # transpose this pair before its y buffer is recycled
                for t4w in range(T4):
                    transp_win(pair, t4w)

        # ---------- tail: remaining transposes + projection ----------
        if full:
            transp_win(3, 2)
            transp_win(3, 3)
            for item in tail_extra:
                item()
            # attention is done: rotate the projection over all 8 psum
            # banks so occ chunks don't serialize on one bank's WAR.
            banks = [den[:, 0:512], sps[0][:, 0:512], sps[0][:, 512:1024],
                     sps[1][:, 0:512], sps[1][:, 512:1024],
                     ups[0][:, 0:512], ups[1][:, 0:512], trans[:, 0:512]]
            bi = 0
            for occ in range(OCC):
                for t4w in (2, 3):
                    proj_occ(occ, t4w, bank=banks[bi % len(banks)])
                    bi += 1
                nc.sync.dma_start(out_t[occ * 128:(occ + 1) * 128, :],
                                  osbs[occ][:])
        else:
            if not wp:
                load_wp()
            for occ in range(OCC):
                for t4w in range(T4):
                    proj_occ(occ, t4w)
                nc.sync.dma_start(out_t[occ * 128:(occ + 1) * 128, :],
                                  osbs[occ][:])
        wv_stack.close()


def build_nc(T=2048):
    FC = 2 * CG // 128
    OCC = C // 128
    nc = bacc.Bacc("TRN2", target_bir_lowering=False, debug=False,
                   num_devices=N_CORES)
    x_t = nc.dram_tensor("x_t", [C, T], BF16, kind="ExternalInput")
    w_qk = nc.dram_tensor("w_qk", [C, 2 * CG], BF16, kind="ExternalInput")
    b_qk = nc.dram_tensor("b_qk", [128, FC], F32, kind="ExternalInput")
    w_v = nc.dram_tensor("w_v", [C, CG], BF16, kind="ExternalInput")
    w_p = nc.dram_tensor("w_p", [CG, C], BF16, kind="ExternalInput")
    b_out = nc.dram_tensor("b_out", [128, OCC], F32, kind="ExternalInput")
    id_bf = nc.dram_tensor("id_bf", [128, 128], BF16, kind="ExternalInput")
    out_t = nc.dram_tensor("out_t", [C, T], BF16, kind="ExternalOutput")
    with tile.TileContext(nc) as tc:
        _body(tc, T, x_t.ap(), w_qk.ap(), b_qk.ap(), w_v.ap(),
              w_p.ap(), b_out.ap(), id_bf.ap(), out_t.ap())
    nc.compile()
    return nc


def shard_inputs(sequences, w_attn, b_attn, w_proj, b_proj):
    """Build the 8 per-core input maps. Core index = b*2 + g."""
    sequences = np.asarray(sequences, dtype=np.float32)
    w_attn = np.asarray(w_attn, dtype=np.float32)
    b_attn = np.asarray(b_attn, dtype=np.float32)
    w_proj = np.asarray(w_proj, dtype=np.float32)
    b_proj = np.asarray(b_proj, dtype=np.float32)
    B = sequences.shape[0]
    ident = np.eye(128, dtype=ml_dtypes.bfloat16)
    in_maps = []
    for b in range(B):
        for g in range(2):
            qs = slice(g * CG, (g + 1) * CG)
            ks = slice(C + g * CG, C + (g + 1) * CG)
            vs = slice(2 * C + g * CG, 2 * C + (g + 1) * CG)
            in_maps.append({
                "x_t": np.ascontiguousarray(sequences[b].T)
                    .astype(ml_dtypes.bfloat16),
                "w_qk": np.ascontiguousarray(
                    np.concatenate([w_attn[:, qs], w_attn[:, ks]], axis=1))
                    .astype(ml_dtypes.bfloat16),
                "b_qk": np.ascontiguousarray(
                    np.concatenate([b_attn[qs], b_attn[ks]])
                    .reshape(8, 128).T),
                "w_v": np.ascontiguousarray(w_attn[:, vs])
                    .astype(ml_dtypes.bfloat16),
                "w_p": np.ascontiguousarray(w_proj[g * CG:(g + 1) * CG, :])
                    .astype(ml_dtypes.bfloat16),
                # softmax rows sum to 1, so the v-bias folds into the output
                # bias: y_g = attn@(x@w_v) @ w_p + (b_v@w_p [+ b_proj on g0])
                "b_out": np.ascontiguousarray(
                    (b_attn[vs] @ w_proj[g * CG:(g + 1) * CG, :]
                     + (b_proj if g == 0 else 0.0))
                    .astype(np.float32).reshape(8, 128).T),
                "id_bf": ident,
            })
    return in_maps


def unshard_outputs(outs, B, T):
    """outs: list of 8 [C, T] partials, core index = b*2+g."""
    y = np.empty((B, T, C), np.float32)
    for b in range(B):
        y[b] = (np.asarray(outs[2 * b], np.float32)
                + np.asarray(outs[2 * b + 1], np.float32)).T
    return y


_NC_CACHE = {}


def kernel(sequences, w_attn, b_attn, w_proj, b_proj):
    sequences = np.asarray(sequences, dtype=np.float32)
    B, T, _ = sequences.shape
    in_maps = shard_inputs(sequences, w_attn, b_attn, w_proj, b_proj)
    if T not in _NC_CACHE:
        _NC_CACHE[T] = build_nc(T)
    nc = _NC_CACHE[T]
    res = run_bass_kernel_spmd(nc, in_maps, list(range(N_CORES)))
    outs = [res.results[i]["out_t"] for i in range(N_CORES)]
    return unshard_outputs(outs, B, T)


if __name__ == "__main__":
    rng = np.random.default_rng(0)
    B, T = 4, 2048
    seq = rng.standard_normal((B, T, C), dtype=np.float32)
    wa = rng.standard_normal((C, 3 * C), dtype=np.float32) / np.sqrt(C)
    ba = np.zeros(3 * C, np.float32)
    wp = rng.standard_normal((C, C), dtype=np.float32) / np.sqrt(C)
    bp = np.zeros(C, np.float32)
    y = kernel(seq, wa, ba, wp, bp)
    print(y.shape, y.dtype)



# revision 2
# speedup vs baseline: 2.3692x; 2.3692x over previous
"""Multi-head self-attention Trainium2 Bass kernel, v3.

Sharding (unchanged from v2): core = (batch b, head-group g of 8 heads);
the host sums the two head-groups' partial projections.

v3 is a ground-up reschedule driven by HW microbenchmarks (the cost
model badly underestimates fine-grained matmul overhead on this part):

- The softmax denominator is folded into the AV matmul: v tiles carry a
  ones column per head (stride 65), so each AV group is ONE matmul of
  N=65 instead of MM(64) + MM(1) + an extra weight reload.  The N=1
  denominator matmuls were ~500ns/group on HW (PSUM zero-region cost);
  the merged form measures ~25ns/group.
- One exp ACT per (pair, win, kc) covering both heads ([128,1024] PSUM
  -> SBUF bf16, ~1.07us measured).  256 ACTs ~= 275us is the kernel's
  hard floor (33.5M exps/core at 1 elem/lane/cycle @1.2GHz); the whole
  schedule exists to keep the ACT queue full.
- Scores ping-pong between two 2-bank PSUM regions so scores(i+1) runs
  while ACT(i) drains; AV(i) is emitted AFTER scores(i+1) so the
  in-order PE queue never parks ACT's inputs behind an et-wait.
- Dense work (QKV projection, PE transposes, output projection) is a
  deque of ~1-2us filler items popped into the per-iteration PE slack.
"""

from collections import deque
from contextlib import ExitStack

import bass_rust as _br
import numpy as np
import ml_dtypes

import concourse.bass as bass
import concourse.bacc as bacc
import concourse.tile as tile
from concourse import mybir
from concourse.bass_utils import run_bass_kernel_spmd

N_CORES = 8
C = 1024           # embed dim
H = 16             # total heads
HD = 64            # head dim
HPC = 8            # heads per core
CG = HPC * HD      # 512: per-core q/k/v feature width
NPAIR = HPC // 2   # 4 head pairs
QW = 512           # query window

F32 = mybir.dt.float32
BF16 = mybir.dt.bfloat16


def _order_after(inst, first):
    deps = _br.InstructionNameOrderedSet()
    deps.add(first.ins.name)
    inst.ins.add_nosync_dependencies_from(deps)


def _body(tc, T, x_t, w_qk, b_qk, w_v, w_p, b_out, id_bf, out_t):
    nc = tc.nc
    KC = C // 128            # 8 contraction chunks over C
    FC = 2 * CG // 128       # 8 q||k feature chunks
    TC1 = T // 128           # key chunks of 128
    NW = T // QW             # query windows per pair
    NQ = QW // 128           # 4 query chunks per window
    OCC = C // 128           # 8 output channel chunks
    PCH = CG // 128          # 4 proj contraction chunks
    Exp = mybir.ActivationFunctionType.Exp
    Mult = mybir.AluOpType.mult

    with ExitStack() as stack:
        constp = stack.enter_context(tc.tile_pool(name="const", bufs=1))
        pers = stack.enter_context(tc.tile_pool(name="persist", bufs=1))
        ep = stack.enter_context(tc.tile_pool(name="e", bufs=6))
        recp = stack.enter_context(tc.tile_pool(name="rec", bufs=2))
        osbp = stack.enter_context(tc.tile_pool(name="osb", bufs=4))
        psp = stack.enter_context(tc.tile_pool(name="ps", bufs=1,
                                               space="PSUM"))

        # ---- constants / small inputs first in DMA queue order
        bqk_sb = constp.tile([128, FC], F32, tag="bqk")
        nc.sync.dma_start(bqk_sb[:], b_qk[:])
        bout_sb = constp.tile([128, OCC], F32, tag="bout")
        nc.sync.dma_start(bout_sb[:], b_out[:])
        id_sb = constp.tile([128, 128], BF16, tag="id")

        # w_qk: all 8 feature chunks in one tile, pair0's chunks (fc 0 and
        # 4) DMA'd first so the upfront QK(pair0) can start immediately.
        wqk_all = pers.tile([128, FC * KC * 128], BF16, tag="wqkall")

        def wqk_t(fc, kc):
            return wqk_all[:, (fc * KC + kc) * 128:(fc * KC + kc) * 128 + 128]

        def load_wqk(fc):
            nc.sync.dma_start(
                wqk_all[:, fc * KC * 128:(fc + 1) * KC * 128]
                .rearrange("p (k c) -> p k c", c=128),
                w_qk[:, fc * 128:(fc + 1) * 128]
                .rearrange("(k p) c -> p k c", p=128))

        load_wqk(0)
        load_wqk(NPAIR)

        # activations [C, T] bf16: first halves -> w_v -> second halves
        TH = T // 2
        xt = []
        for i in range(KC):
            t = pers.tile([128, T], BF16, tag=f"xt{i}")
            nc.sync.dma_start(t[:, 0:TH], x_t[i * 128:(i + 1) * 128, 0:TH])
            xt.append(t)
        wv_all = pers.tile([128, KC * CG], BF16, tag="wvall")
        nc.sync.dma_start(
            wv_all.rearrange("p (k c) -> p k c", c=CG),
            w_v.rearrange("(k p) c -> p k c", p=128))
        wv = [wv_all[:, kc * CG:(kc + 1) * CG] for kc in range(KC)]
        nc.sync.dma_start(id_sb[:], id_bf[:])
        for fc in range(FC):
            if fc not in (0, NPAIR):
                load_wqk(fc)
        for i in range(KC):
            nc.sync.dma_start(xt[i][:, TH:T], x_t[i * 128:(i + 1) * 128, TH:T])
        wp_all = pers.tile([128, PCH * C], BF16, tag="wpall")
        nc.sync.dma_start(
            wp_all.rearrange("p (f c) -> p f c", c=C),
            w_p.rearrange("(f p) c -> p f c", p=128))
        wp = [wp_all[:, fcp * C:(fcp + 1) * C] for fcp in range(PCH)]

        # ---- persistent intermediates
        qkt = [pers.tile([128, T], BF16, tag=f"qkt{i}", name=f"qkt{i}")
               for i in range(FC)]
        # v2[tokc]: [128 tok, 8 heads x 65]; col h*65+64 stays 1.0 (the
        # memset) so the AV matmul's 65th output column is the softmax
        # denominator.
        v2 = [pers.tile([128, HPC * 65], BF16, tag=f"v2_{i}", name=f"v2_{i}")
              for i in range(TC1)]
        for t in v2:
            nc.vector.memset(t[:], 1.0)
        # ysb[pair][s]: [128 tok-part, (T//512) * 4qi * 64] normalized y
        ysb = [[pers.tile([128, NW * NQ * 64], BF16, tag=f"y{p}_{s}",
                           name=f"y{p}_{s}") for s in range(2)]
               for p in range(NPAIR)]
        # yt[pair]: [128 feat, T] transposed for the projection
        yt = [pers.tile([128, T], BF16, tag=f"yt{i}", name=f"yt{i}")
              for i in range(NPAIR)]

        # ---- PSUM: 2 score regions (2 banks each) + 2 ups + 2 dense
        sreg = [psp.tile([128, 1024], F32, tag=f"sreg{i}", name=f"sreg{i}")
                for i in range(2)]
        ups = [psp.tile([128, 512], F32, tag=f"ups{s}", name=f"ups{s}")
               for s in range(2)]
        dbank = [psp.tile([128, 512], F32, tag=f"d{i}", name=f"d{i}")
                 for i in range(2)]
        dsel = [0]

        def next_d():
            dsel[0] ^= 1
            return dbank[dsel[0]]

        # ---------- dense work items ----------
        def qk_item(fc, t4w, half):
            ps = dbank[(fc + t4w) % 2]
            for kc in range(half * 4, half * 4 + 4):
                nc.tensor.matmul(
                    ps[:], wqk_t(fc, kc),
                    xt[kc][:, t4w * 512:(t4w + 1) * 512],
                    start=(kc == 0), stop=(kc == KC - 1))
            if half == 1:
                nc.vector.tensor_scalar_add(
                    qkt[fc][:, t4w * 512:(t4w + 1) * 512], ps[:],
                    bqk_sb[:, fc:fc + 1])

        def v_item(tokc, half):
            ps = dbank[tokc % 2]
            for kc in range(half * 4, half * 4 + 4):
                nc.tensor.matmul(
                    ps[:], xt[kc][:, tokc * 128:(tokc + 1) * 128],
                    wv[kc], start=(kc == 0), stop=(kc == KC - 1))
            if half == 1:
                nc.vector.tensor_copy(
                    v2[tokc].rearrange("p (h c) -> p h c", c=65)[:, :, 0:64],
                    ps.rearrange("p (h c) -> p h c", c=64)[:])

        def transp_item(pair, t4w):
            ps = next_d()
            tp = ps.bitcast(BF16)
            yv = [ysb[pair][s].rearrange("p (t c) -> p t c", c=64)
                  for s in range(2)]
            for s in range(2):
                for j in range(NQ):
                    tc_idx = t4w * NQ + j
                    nc.tensor.matmul(
                        tp[s * 64:(s + 1) * 64, j * 128:(j + 1) * 128],
                        yv[s][:, tc_idx, :], id_sb[:],
                        start=True, stop=True, is_transpose=True)
            nc.vector.tensor_copy(
                yt[pair][:, t4w * 512:(t4w + 1) * 512], tp[:, 0:512])

        def proj_item(occ, t4w):
            ps = next_d()
            for fcp in range(PCH):
                nc.tensor.matmul(
                    ps[:], wp[fcp][:, occ * 128:(occ + 1) * 128],
                    yt[fcp][:, t4w * 512:(t4w + 1) * 512],
                    start=(fcp == 0), stop=(fcp == PCH - 1))
            osb = osbp.tile([128, 512], BF16, tag="osb")
            nc.vector.tensor_scalar_add(osb[:], ps[:],
                                        bout_sb[:, occ:occ + 1])
            nc.sync.dma_start(
                out_t[occ * 128:(occ + 1) * 128,
                      t4w * 512:(t4w + 1) * 512], osb[:])

        # ---------- iteration stream ----------
        iters = [(pair, win, kc) for pair in range(NPAIR)
                 for win in range(NW) for kc in range(TC1)]
        NIT = len(iters)
        et_handles = {}

        def emit_scores_act(j):
            pair, win, kc = iters[j]
            reg = sreg[j % 2]
            q0 = win * QW
            qt, kt = qkt[pair], qkt[NPAIR + pair]
            for s in range(2):
                po = s * 64
                nc.tensor.matmul(
                    reg[:, s * 512:(s + 1) * 512],
                    kt[po:po + 64, kc * 128:(kc + 1) * 128],
                    qt[po:po + 64, q0:q0 + QW],
                    start=True, stop=True)
            et = ep.tile([128, 1024], BF16, tag="et", name=f"et_{j}")
            nc.scalar.activation(et[:], reg[:], Exp, scale=0.125)
            et_handles[j] = et

        def emit_av(j):
            pair, win, kc = iters[j]
            et = et_handles.pop(j)
            first, last = (kc == 0), (kc == TC1 - 1)
            # one start per ups bank per window: start zeroes the full 2KB
            # zero region.  The qi>0 kc==0 writes must execute after the
            # start MM; they share its deps (same et tile, same WAR on the
            # previous window's normalize), so the scheduler's priority
            # heap preserves emission order — no explicit edges needed
            # (explicit nosync edges here measured 10x slower on HW).
            for s in range(2):
                h = 2 * pair + s
                for qi in range(NQ):
                    nc.tensor.matmul(
                        ups[s][:, qi * 65:qi * 65 + 65],
                        et[:, s * 512 + qi * 128:s * 512 + qi * 128 + 128],
                        v2[kc][:, h * 65:h * 65 + 65],
                        start=(first and qi == 0),
                        stop=(last and qi == NQ - 1))

        def emit_norm(pair, win):
            rden = recp.tile([128, 2 * NQ], F32, tag="rden",
                             name=f"rden_{pair}_{win}")
            for s in range(2):
                uv = ups[s][:, 0:NQ * 65].rearrange("p (q c) -> p q c", c=65)
                nc.vector.reciprocal(
                    rden[:, s * NQ:(s + 1) * NQ].unsqueeze(2),
                    uv[:, :, 64:65])
                yv = ysb[pair][s].rearrange("p (w q d) -> p w q d",
                                            w=NW, d=64)
                nc.vector.tensor_tensor(
                    yv[:, win, :, :], uv[:, :, 0:64],
                    rden[:, s * NQ:(s + 1) * NQ].unsqueeze(2)
                    .to_broadcast((128, NQ, 64)),
                    op=Mult)

        # ---------- filler queue: (ready_iter, deadline_iter, fn) ----------
        # ready: don't pop before this iteration (the item's inputs exist).
        # deadline: MUST be emitted before this iteration's scores/AV (the
        # scheduler keeps per-engine emission order, so a consumer emitted
        # before its producer reads garbage).
        NIT = NPAIR * NW * TC1
        queue = deque()
        for tokc in range(2, TC1):
            for hf in range(2):
                # v2[tokc] is read by emit_av at iteration tokc
                queue.append((0, tokc, lambda tokc=tokc, hf=hf:
                              v_item(tokc, hf)))
        for p in range(1, NPAIR):
            for fc in (p, NPAIR + p):
                for t4w in range(NW):
                    for hf in range(2):
                        # qkt[fc] read by scores of pair p (emitted at
                        # iteration p*NW*TC1 - 1 via the j+1 lookahead)
                        queue.append(
                            (0, p * NW * TC1 - 1,
                             lambda fc=fc, t4w=t4w, hf=hf:
                             qk_item(fc, t4w, hf)))
        # transposes: (p, t4w) ready after iteration (p, win=t4w, kc last);
        # proj(occ, t4w) ready after the LAST pair's window t4w.
        tr_pr = []
        for t4w in range(NW):
            for p in range(NPAIR):
                rdy = (p * NW + t4w) * TC1 + TC1
                tr_pr.append((rdy, 0, lambda p=p, t4w=t4w:
                              transp_item(p, t4w)))
        for t4w in range(NW):
            rdy = ((NPAIR - 1) * NW + t4w) * TC1 + TC1
            for occ in range(OCC):
                tr_pr.append((rdy, 1, lambda occ=occ, t4w=t4w:
                              proj_item(occ, t4w)))
        tr_pr.sort(key=lambda x: (x[0], x[1]))
        for rdy, _, fn in tr_pr:
            queue.append((rdy, NIT + 1, fn))

        # ---------- upfront: QK(pair0) + first two V chunks ----------
        for fc in (0, NPAIR):
            for t4w in range(NW):
                qk_item(fc, t4w, 0)
                qk_item(fc, t4w, 1)
        for tokc in range(min(2, TC1)):
            v_item(tokc, 0)
            v_item(tokc, 1)

        # ---------- main loop ----------
        emit_scores_act(0)
        for j in range(NIT):
            # deadline items first (correctness), then budgeted fillers
            while queue and queue[0][1] <= j + 1:
                queue.popleft()[2]()
            if j + 1 < NIT:
                emit_scores_act(j + 1)
            emit_av(j)
            pair, win, kc = iters[j]
            if kc == TC1 - 1:
                emit_norm(pair, win)
            budget = 2
            while budget and queue and queue[0][0] <= j + 1 \
                    and queue[0][1] > j + 1:
                queue.popleft()[2]()
                budget -= 1
        while queue:
            queue.popleft()[2]()


def build_nc(T=2048):
    FC = 2 * CG // 128
    OCC = C // 128
    nc = bacc.Bacc("TRN2", target_bir_lowering=False, debug=False,
                   num_devices=N_CORES)
    x_t = nc.dram_tensor("x_t", [C, T], BF16, kind="ExternalInput")
    w_qk = nc.dram_tensor("w_qk", [C, 2 * CG], BF16, kind="ExternalInput")
    b_qk = nc.dram_tensor("b_qk", [128, FC], F32, kind="ExternalInput")
    w_v = nc.dram_tensor("w_v", [C, CG], BF16, kind="ExternalInput")
    w_p = nc.dram_tensor("w_p", [CG, C], BF16, kind="ExternalInput")
    b_out = nc.dram_tensor("b_out", [128, OCC], F32, kind="ExternalInput")
    id_bf = nc.dram_tensor("id_bf", [128, 128], BF16, kind="ExternalInput")
    out_t = nc.dram_tensor("out_t", [C, T], BF16, kind="ExternalOutput")
    with tile.TileContext(nc) as tc:
        _body(tc, T, x_t.ap(), w_qk.ap(), b_qk.ap(), w_v.ap(),
              w_p.ap(), b_out.ap(), id_bf.ap(), out_t.ap())
    nc.compile()
    return nc


def shard_inputs(sequences, w_attn, b_attn, w_proj, b_proj):
    """Build the 8 per-core input maps. Core index = b*2 + g."""
    sequences = np.asarray(sequences, dtype=np.float32)
    w_attn = np.asarray(w_attn, dtype=np.float32)
    b_attn = np.asarray(b_attn, dtype=np.float32)
    w_proj = np.asarray(w_proj, dtype=np.float32)
    b_proj = np.asarray(b_proj, dtype=np.float32)
    B = sequences.shape[0]
    ident = np.eye(128, dtype=ml_dtypes.bfloat16)
    in_maps = []
    for b in range(B):
        for g in range(2):
            qs = slice(g * CG, (g + 1) * CG)
            ks = slice(C + g * CG, C + (g + 1) * CG)
            vs = slice(2 * C + g * CG, 2 * C + (g + 1) * CG)
            in_maps.append({
                "x_t": np.ascontiguousarray(sequences[b].T)
                    .astype(ml_dtypes.bfloat16),
                "w_qk": np.ascontiguousarray(
                    np.concatenate([w_attn[:, qs], w_attn[:, ks]], axis=1))
                    .astype(ml_dtypes.bfloat16),
                "b_qk": np.ascontiguousarray(
                    np.concatenate([b_attn[qs], b_attn[ks]])
                    .reshape(8, 128).T),
                "w_v": np.ascontiguousarray(w_attn[:, vs])
                    .astype(ml_dtypes.bfloat16),
                "w_p": np.ascontiguousarray(w_proj[g * CG:(g + 1) * CG, :])
                    .astype(ml_dtypes.bfloat16),
                # softmax rows sum to 1, so the v-bias folds into the output
                # bias: y_g = attn@(x@w_v) @ w_p + (b_v@w_p [+ b_proj on g0])
                "b_out": np.ascontiguousarray(
                    (b_attn[vs] @ w_proj[g * CG:(g + 1) * CG, :]
                     + (b_proj if g == 0 else 0.0))
                    .astype(np.float32).reshape(8, 128).T),
                "id_bf": ident,
            })
    return in_maps


def unshard_outputs(outs, B, T):
    """outs: list of 8 [C, T] partials, core index = b*2+g."""
    y = np.empty((B, T, C), np.float32)
    for b in range(B):
        y[b] = (np.asarray(outs[2 * b], np.float32)
                + np.asarray(outs[2 * b + 1], np.float32)).T
    return y


_NC_CACHE = {}


def kernel(sequences, w_attn, b_attn, w_proj, b_proj):
    sequences = np.asarray(sequences, dtype=np.float32)
    B, T, _ = sequences.shape
    in_maps = shard_inputs(sequences, w_attn, b_attn, w_proj, b_proj)
    if T not in _NC_CACHE:
        _NC_CACHE[T] = build_nc(T)
    nc = _NC_CACHE[T]
    res = run_bass_kernel_spmd(nc, in_maps, list(range(N_CORES)))
    outs = [res.results[i]["out_t"] for i in range(N_CORES)]
    return unshard_outputs(outs, B, T)


if __name__ == "__main__":
    rng = np.random.default_rng(0)
    B, T = 4, 2048
    seq = rng.standard_normal((B, T, C), dtype=np.float32)
    wa = rng.standard_normal((C, 3 * C), dtype=np.float32) / np.sqrt(C)
    ba = np.zeros(3 * C, np.float32)
    wpj = rng.standard_normal((C, C), dtype=np.float32) / np.sqrt(C)
    bp = np.zeros(C, np.float32)
    y = kernel(seq, wa, ba, wpj, bp)
    print(y.shape, y.dtype)


# revision 4
# speedup vs baseline: 3.4920x; 1.4739x over previous
"""Multi-head self-attention Trainium2 Bass kernel, v3.

Sharding (unchanged from v2): core = (batch b, head-group g of 8 heads);
the host sums the two head-groups' partial projections.

v3 is a ground-up reschedule driven by HW microbenchmarks (the cost
model badly underestimates fine-grained matmul overhead on this part):

- The softmax denominator is folded into the AV matmul: v tiles carry a
  ones column per head (stride 65), so each AV group is ONE matmul of
  N=65 instead of MM(64) + MM(1) + an extra weight reload.  The N=1
  denominator matmuls were ~500ns/group on HW (PSUM zero-region cost);
  the merged form measures ~25ns/group.
- One exp ACT per (pair, win, kc) covering both heads ([128,1024] PSUM
  -> SBUF bf16, ~1.07us measured).  256 ACTs ~= 275us is the kernel's
  hard floor (33.5M exps/core at 1 elem/lane/cycle @1.2GHz); the whole
  schedule exists to keep the ACT queue full.
- Scores ping-pong between two 2-bank PSUM regions so scores(i+1) runs
  while ACT(i) drains; AV(i) is emitted AFTER scores(i+1) so the
  in-order PE queue never parks ACT's inputs behind an et-wait.
- Dense work (QKV projection, PE transposes, output projection) is a
  deque of ~1-2us filler items popped into the per-iteration PE slack.
"""

from collections import deque
from contextlib import ExitStack

import bass_rust as _br
import numpy as np
import ml_dtypes

import concourse.bass as bass
import concourse.bacc as bacc
import concourse.tile as tile
from concourse import mybir
from concourse.bass_utils import run_bass_kernel_spmd

N_CORES = 8
C = 1024           # embed dim
H = 16             # total heads
HD = 64            # head dim
HPC = 8            # heads per core
CG = HPC * HD      # 512: per-core q/k/v feature width
NPAIR = HPC // 2   # 4 head pairs
QW = 512           # query window

F32 = mybir.dt.float32
BF16 = mybir.dt.bfloat16


def _order_after(inst, first):
    deps = _br.InstructionNameOrderedSet()
    deps.add(first.ins.name)
    inst.ins.add_nosync_dependencies_from(deps)


def _body(tc, T, x_t, w_qk, b_qk, w_v, w_p, b_out, id_bf, out_t):
    nc = tc.nc
    KC = C // 128            # 8 contraction chunks over C
    FC = 2 * CG // 128       # 8 q||k feature chunks
    TC1 = T // 128           # key chunks of 128
    NW = T // QW             # query windows per pair
    NQ = QW // 128           # 4 query chunks per window
    OCC = C // 128           # 8 output channel chunks
    PCH = CG // 128          # 4 proj contraction chunks
    Exp = mybir.ActivationFunctionType.Exp
    Mult = mybir.AluOpType.mult

    with ExitStack() as stack:
        constp = stack.enter_context(tc.tile_pool(name="const", bufs=1))
        pers = stack.enter_context(tc.tile_pool(name="persist", bufs=1))
        ep = stack.enter_context(tc.tile_pool(name="e", bufs=6))
        recp = stack.enter_context(tc.tile_pool(name="rec", bufs=2))
        osbp = stack.enter_context(tc.tile_pool(name="osb", bufs=4))
        psp = stack.enter_context(tc.tile_pool(name="ps", bufs=1,
                                               space="PSUM"))

        # ---- constants / small inputs first in DMA queue order
        bqk_sb = constp.tile([128, FC], F32, tag="bqk")
        nc.sync.dma_start(bqk_sb[:], b_qk[:])
        bout_sb = constp.tile([128, OCC], F32, tag="bout")
        nc.sync.dma_start(bout_sb[:], b_out[:])
        id_sb = constp.tile([128, 128], BF16, tag="id")

        # w_qk: all 8 feature chunks in one tile, pair0's chunks (fc 0 and
        # 4) DMA'd first so the upfront QK(pair0) can start immediately.
        wqk_all = pers.tile([128, FC * KC * 128], BF16, tag="wqkall")

        def wqk_t(fc, kc):
            return wqk_all[:, (fc * KC + kc) * 128:(fc * KC + kc) * 128 + 128]

        def load_wqk(fc):
            nc.sync.dma_start(
                wqk_all[:, fc * KC * 128:(fc + 1) * KC * 128]
                .rearrange("p (k c) -> p k c", c=128),
                w_qk[:, fc * 128:(fc + 1) * 128]
                .rearrange("(k p) c -> p k c", p=128))

        load_wqk(0)
        load_wqk(NPAIR)

        # activations [C, T] bf16: first halves -> w_v -> second halves
        TH = T // 2
        xt = []
        for i in range(KC):
            t = pers.tile([128, T], BF16, tag=f"xt{i}")
            nc.sync.dma_start(t[:, 0:TH], x_t[i * 128:(i + 1) * 128, 0:TH])
            xt.append(t)
        wv_all = pers.tile([128, KC * CG], BF16, tag="wvall")
        nc.sync.dma_start(
            wv_all.rearrange("p (k c) -> p k c", c=CG),
            w_v.rearrange("(k p) c -> p k c", p=128))
        wv = [wv_all[:, kc * CG:(kc + 1) * CG] for kc in range(KC)]
        nc.sync.dma_start(id_sb[:], id_bf[:])
        for fc in range(FC):
            if fc not in (0, NPAIR):
                load_wqk(fc)
        for i in range(KC):
            nc.sync.dma_start(xt[i][:, TH:T], x_t[i * 128:(i + 1) * 128, TH:T])
        wp_all = pers.tile([128, PCH * C], BF16, tag="wpall")
        nc.sync.dma_start(
            wp_all.rearrange("p (f c) -> p f c", c=C),
            w_p.rearrange("(f p) c -> p f c", p=128))
        wp = [wp_all[:, fcp * C:(fcp + 1) * C] for fcp in range(PCH)]

        # ---- persistent intermediates
        qkt = [pers.tile([128, T], BF16, tag=f"qkt{i}", name=f"qkt{i}")
               for i in range(FC)]
        # v2[tokc]: [128 tok, 8 heads x 65]; col h*65+64 stays 1.0 (the
        # memset) so the AV matmul's 65th output column is the softmax
        # denominator.
        v2 = [pers.tile([128, HPC * 65], BF16, tag=f"v2_{i}", name=f"v2_{i}")
              for i in range(TC1)]
        for t in v2:
            nc.vector.memset(t[:], 1.0)
        # ysb[pair][s]: [128 tok-part, (T//512) * 4qi * 64] normalized y
        ysb = [[pers.tile([128, NW * NQ * 64], BF16, tag=f"y{p}_{s}",
                           name=f"y{p}_{s}") for s in range(2)]
               for p in range(NPAIR)]
        # yt[pair]: [128 feat, T] transposed for the projection
        yt = [pers.tile([128, T], BF16, tag=f"yt{i}", name=f"yt{i}")
              for i in range(NPAIR)]

        # ---- PSUM: 2 score regions (2 banks each) + 2 ups + 2 dense
        sreg = [psp.tile([128, 1024], F32, tag=f"sreg{i}", name=f"sreg{i}")
                for i in range(2)]
        ups = [psp.tile([128, 512], F32, tag=f"ups{s}", name=f"ups{s}")
               for s in range(2)]
        dbank = [psp.tile([128, 512], F32, tag=f"d{i}", name=f"d{i}")
                 for i in range(2)]
        dsel = [0]

        def next_d():
            dsel[0] ^= 1
            return dbank[dsel[0]]

        # ---------- dense work items ----------
        def qk_item(fc, t4w, half):
            ps = dbank[(fc + t4w) % 2]
            for kc in range(half * 4, half * 4 + 4):
                nc.tensor.matmul(
                    ps[:], wqk_t(fc, kc),
                    xt[kc][:, t4w * 512:(t4w + 1) * 512],
                    start=(kc == 0), stop=(kc == KC - 1))
            if half == 1:
                nc.vector.tensor_scalar_add(
                    qkt[fc][:, t4w * 512:(t4w + 1) * 512], ps[:],
                    bqk_sb[:, fc:fc + 1])

        def v_item(tokc, half):
            ps = dbank[tokc % 2]
            for kc in range(half * 4, half * 4 + 4):
                nc.tensor.matmul(
                    ps[:], xt[kc][:, tokc * 128:(tokc + 1) * 128],
                    wv[kc], start=(kc == 0), stop=(kc == KC - 1))
            if half == 1:
                nc.vector.tensor_copy(
                    v2[tokc].rearrange("p (h c) -> p h c", c=65)[:, :, 0:64],
                    ps.rearrange("p (h c) -> p h c", c=64)[:])

        def transp_item(pair, t4w):
            ps = next_d()
            tp = ps.bitcast(BF16)
            yv = [ysb[pair][s].rearrange("p (t c) -> p t c", c=64)
                  for s in range(2)]
            for s in range(2):
                for j in range(NQ):
                    tc_idx = t4w * NQ + j
                    nc.tensor.matmul(
                        tp[s * 64:(s + 1) * 64, j * 128:(j + 1) * 128],
                        yv[s][:, tc_idx, :], id_sb[:],
                        start=True, stop=True, is_transpose=True)
            nc.vector.tensor_copy(
                yt[pair][:, t4w * 512:(t4w + 1) * 512], tp[:, 0:512])

        def proj_item(occ, t4w):
            ps = next_d()
            for fcp in range(PCH):
                nc.tensor.matmul(
                    ps[:], wp[fcp][:, occ * 128:(occ + 1) * 128],
                    yt[fcp][:, t4w * 512:(t4w + 1) * 512],
                    start=(fcp == 0), stop=(fcp == PCH - 1))
            osb = osbp.tile([128, 512], BF16, tag="osb")
            nc.vector.tensor_scalar_add(osb[:], ps[:],
                                        bout_sb[:, occ:occ + 1])
            nc.sync.dma_start(
                out_t[occ * 128:(occ + 1) * 128,
                      t4w * 512:(t4w + 1) * 512], osb[:])

        # ---------- iteration stream ----------
        iters = [(pair, win, kc) for pair in range(NPAIR)
                 for win in range(NW) for kc in range(TC1)]
        NIT = len(iters)
        et_handles = {}

        def emit_scores_act(j):
            pair, win, kc = iters[j]
            reg = sreg[j % 2]
            q0 = win * QW
            qt, kt = qkt[pair], qkt[NPAIR + pair]
            for s in range(2):
                po = s * 64
                nc.tensor.matmul(
                    reg[:, s * 512:(s + 1) * 512],
                    kt[po:po + 64, kc * 128:(kc + 1) * 128],
                    qt[po:po + 64, q0:q0 + QW],
                    start=True, stop=True)
            et = ep.tile([128, 1024], BF16, tag="et", name=f"et_{j}")
            nc.scalar.activation(et[:], reg[:], Exp, scale=0.125)
            et_handles[j] = et

        def emit_av(j):
            pair, win, kc = iters[j]
            et = et_handles.pop(j)
            first, last = (kc == 0), (kc == TC1 - 1)
            # one start per ups bank per window: start zeroes the full 2KB
            # zero region.  The qi>0 kc==0 writes must execute after the
            # start MM; they share its deps (same et tile, same WAR on the
            # previous window's normalize), so the scheduler's priority
            # heap preserves emission order — no explicit edges needed
            # (explicit nosync edges here measured 10x slower on HW).
            for s in range(2):
                h = 2 * pair + s
                for qi in range(NQ):
                    nc.tensor.matmul(
                        ups[s][:, qi * 65:qi * 65 + 65],
                        et[:, s * 512 + qi * 128:s * 512 + qi * 128 + 128],
                        v2[kc][:, h * 65:h * 65 + 65],
                        start=(first and qi == 0),
                        stop=(last and qi == NQ - 1))

        def emit_norm(pair, win):
            rden = recp.tile([128, 2 * NQ], F32, tag="rden",
                             name=f"rden_{pair}_{win}")
            for s in range(2):
                uv = ups[s][:, 0:NQ * 65].rearrange("p (q c) -> p q c", c=65)
                nc.vector.reciprocal(
                    rden[:, s * NQ:(s + 1) * NQ].unsqueeze(2),
                    uv[:, :, 64:65])
                yv = ysb[pair][s].rearrange("p (w q d) -> p w q d",
                                            w=NW, d=64)
                nc.vector.tensor_tensor(
                    yv[:, win, :, :], uv[:, :, 0:64],
                    rden[:, s * NQ:(s + 1) * NQ].unsqueeze(2)
                    .to_broadcast((128, NQ, 64)),
                    op=Mult)

        # ---------- filler queue: (ready_iter, deadline_iter, fn) ----------
        # ready: don't pop before this iteration (the item's inputs exist).
        # deadline: MUST be emitted before this iteration's scores/AV (the
        # scheduler keeps per-engine emission order, so a consumer emitted
        # before its producer reads garbage).
        NIT = NPAIR * NW * TC1
        queue = deque()
        for tokc in range(2, TC1):
            for hf in range(2):
                # v2[tokc] is read by emit_av at iteration tokc
                queue.append((0, tokc, lambda tokc=tokc, hf=hf:
                              v_item(tokc, hf)))
        for p in range(1, NPAIR):
            for fc in (p, NPAIR + p):
                for t4w in range(NW):
                    for hf in range(2):
                        # qkt[fc] read by scores of pair p (emitted at
                        # iteration p*NW*TC1 - 1 via the j+1 lookahead)
                        queue.append(
                            (0, p * NW * TC1 - 1,
                             lambda fc=fc, t4w=t4w, hf=hf:
                             qk_item(fc, t4w, hf)))
        # transposes: (p, t4w) ready after iteration (p, win=t4w, kc last);
        # proj(occ, t4w) ready after the LAST pair's window t4w.
        # pops now happen at iteration START, before that iteration's norm:
        # window (p, w) ends at iteration e = (p*NW+w)*TC1 + TC1 - 1 and its
        # norm is emitted at the END of e, so consumers are ready at e+2.
        tr_pr = []
        for t4w in range(NW):
            for p in range(NPAIR):
                rdy = (p * NW + t4w) * TC1 + TC1 + 1
                tr_pr.append((rdy, 0, lambda p=p, t4w=t4w:
                              transp_item(p, t4w)))
        for t4w in range(NW):
            rdy = ((NPAIR - 1) * NW + t4w) * TC1 + TC1 + 1
            for occ in range(OCC):
                tr_pr.append((rdy, 1, lambda occ=occ, t4w=t4w:
                              proj_item(occ, t4w)))
        tr_pr.sort(key=lambda x: (x[0], x[1]))
        for rdy, _, fn in tr_pr:
            queue.append((rdy, NIT + 1, fn))

        # ---------- upfront: QK(pair0) + first two V chunks ----------
        for fc in (0, NPAIR):
            for t4w in range(NW):
                qk_item(fc, t4w, 0)
                qk_item(fc, t4w, 1)
        for tokc in range(min(2, TC1)):
            v_item(tokc, 0)
            v_item(tokc, 1)

        # ---------- main loop ----------
        # All filler pops are emitted BEFORE scores(j+1): the PE is gated
        # on ACT completions at scores (PSUM WAR) and at av (et read), so
        # work placed after those gates lands in the scores<->ACT critical
        # cycle and directly stretches the period.
        emit_scores_act(0)
        for j in range(NIT):
            # deadline items first (correctness), then one budgeted filler
            while queue and queue[0][1] <= j + 1:
                queue.popleft()[2]()
            if queue and queue[0][0] <= j + 1:
                queue.popleft()[2]()
            if j + 1 < NIT:
                emit_scores_act(j + 1)
            emit_av(j)
            pair, win, kc = iters[j]
            if kc == TC1 - 1:
                emit_norm(pair, win)
        while queue:
            queue.popleft()[2]()


def build_nc(T=2048):
    FC = 2 * CG // 128
    OCC = C // 128
    nc = bacc.Bacc("TRN2", target_bir_lowering=False, debug=False,
                   num_devices=N_CORES)
    x_t = nc.dram_tensor("x_t", [C, T], BF16, kind="ExternalInput")
    w_qk = nc.dram_tensor("w_qk", [C, 2 * CG], BF16, kind="ExternalInput")
    b_qk = nc.dram_tensor("b_qk", [128, FC], F32, kind="ExternalInput")
    w_v = nc.dram_tensor("w_v", [C, CG], BF16, kind="ExternalInput")
    w_p = nc.dram_tensor("w_p", [CG, C], BF16, kind="ExternalInput")
    b_out = nc.dram_tensor("b_out", [128, OCC], F32, kind="ExternalInput")
    id_bf = nc.dram_tensor("id_bf", [128, 128], BF16, kind="ExternalInput")
    out_t = nc.dram_tensor("out_t", [C, T], BF16, kind="ExternalOutput")
    with tile.TileContext(nc) as tc:
        _body(tc, T, x_t.ap(), w_qk.ap(), b_qk.ap(), w_v.ap(),
              w_p.ap(), b_out.ap(), id_bf.ap(), out_t.ap())
    nc.compile()
    return nc


def shard_inputs(sequences, w_attn, b_attn, w_proj, b_proj):
    """Build the 8 per-core input maps. Core index = b*2 + g."""
    sequences = np.asarray(sequences, dtype=np.float32)
    w_attn = np.asarray(w_attn, dtype=np.float32)
    b_attn = np.asarray(b_attn, dtype=np.float32)
    w_proj = np.asarray(w_proj, dtype=np.float32)
    b_proj = np.asarray(b_proj, dtype=np.float32)
    B = sequences.shape[0]
    ident = np.eye(128, dtype=ml_dtypes.bfloat16)
    in_maps = []
    for b in range(B):
        for g in range(2):
            qs = slice(g * CG, (g + 1) * CG)
            ks = slice(C + g * CG, C + (g + 1) * CG)
            vs = slice(2 * C + g * CG, 2 * C + (g + 1) * CG)
            in_maps.append({
                "x_t": np.ascontiguousarray(sequences[b].T)
                    .astype(ml_dtypes.bfloat16),
                "w_qk": np.ascontiguousarray(
                    np.concatenate([w_attn[:, qs], w_attn[:, ks]], axis=1))
                    .astype(ml_dtypes.bfloat16),
                "b_qk": np.ascontiguousarray(
                    np.concatenate([b_attn[qs], b_attn[ks]])
                    .reshape(8, 128).T),
                "w_v": np.ascontiguousarray(w_attn[:, vs])
                    .astype(ml_dtypes.bfloat16),
                "w_p": np.ascontiguousarray(w_proj[g * CG:(g + 1) * CG, :])
                    .astype(ml_dtypes.bfloat16),
                # softmax rows sum to 1, so the v-bias folds into the output
                # bias: y_g = attn@(x@w_v) @ w_p + (b_v@w_p [+ b_proj on g0])
                "b_out": np.ascontiguousarray(
                    (b_attn[vs] @ w_proj[g * CG:(g + 1) * CG, :]
                     + (b_proj if g == 0 else 0.0))
                    .astype(np.float32).reshape(8, 128).T),
                "id_bf": ident,
            })
    return in_maps


def unshard_outputs(outs, B, T):
    """outs: list of 8 [C, T] partials, core index = b*2+g."""
    y = np.empty((B, T, C), np.float32)
    for b in range(B):
        y[b] = (np.asarray(outs[2 * b], np.float32)
                + np.asarray(outs[2 * b + 1], np.float32)).T
    return y


_NC_CACHE = {}


def kernel(sequences, w_attn, b_attn, w_proj, b_proj):
    sequences = np.asarray(sequences, dtype=np.float32)
    B, T, _ = sequences.shape
    in_maps = shard_inputs(sequences, w_attn, b_attn, w_proj, b_proj)
    if T not in _NC_CACHE:
        _NC_CACHE[T] = build_nc(T)
    nc = _NC_CACHE[T]
    res = run_bass_kernel_spmd(nc, in_maps, list(range(N_CORES)))
    outs = [res.results[i]["out_t"] for i in range(N_CORES)]
    return unshard_outputs(outs, B, T)


if __name__ == "__main__":
    rng = np.random.default_rng(0)
    B, T = 4, 2048
    seq = rng.standard_normal((B, T, C), dtype=np.float32)
    wa = rng.standard_normal((C, 3 * C), dtype=np.float32) / np.sqrt(C)
    ba = np.zeros(3 * C, np.float32)
    wpj = rng.standard_normal((C, C), dtype=np.float32) / np.sqrt(C)
    bp = np.zeros(C, np.float32)
    y = kernel(seq, wa, ba, wpj, bp)
    print(y.shape, y.dtype)
